# revision 6
# baseline (speedup 1.0000x reference)
"""MoE (8 experts, top-2) Trainium2 kernel, expert-parallel across 8 NeuronCores.

Strategy:
  - Each core owns one expert (weights sharded along the expert axis; gate
    replicated). Everything data-dependent runs on device:
      * router logits (fp32 matmul), top-2 + renormalized gate weights (DVE)
      * per-expert token compaction (gpsimd sparse_gather)
      * token dispatch (indirect DMA gather of selected token rows)
      * expert MLP GEMMs in fp32r (silu(x@w1) * (x@w3)) @ w2, scaled by the
        gate coefficient
  - Each core returns its expert's (transposed) token outputs + the compacted
    token index list; the host scatter-adds the 8 partial outputs (the
    "combine" / unshard step).
"""
import sys

sys.path.insert(0, "/opt/trn_rl_repo")

import numpy as np

T, H, II, E = 2048, 1024, 4096, 8
P = 128
NT = T // P          # 16 token tiles
HC = H // P          # 8 hidden chunks
IC = II // P         # 32 intermediate chunks
NCORES = 8

_build_cache = {}


def _build(cap):
    """Build + schedule the per-core Tile kernel for token capacity `cap`."""
    import concourse.bass as bass
    import concourse.bacc as bacc
    import concourse.mybir as mybir
    from concourse.tile import TileContext

    f32 = mybir.dt.float32
    f32r = mybir.dt.float32r
    i32 = mybir.dt.int32
    u32 = mybir.dt.uint32
    u8 = mybir.dt.uint8
    AF = mybir.ActivationFunctionType
    OP = mybir.AluOpType

    _grp_table = {512: 2, 640: 2, 768: 2, 896: 2, 1024: 2, 1536: 3, 2048: 4}
    assert cap in _grp_table, cap
    ngrp = _grp_table[cap]
    grp = cap // ngrp    # token group size per PSUM accumulation (<=512, >=256)
    cf = cap // 16       # free cols of [16, cf] compacted layout
    ntt = cap // P       # gather tiles

    nc = bacc.Bacc("TRN2", target_bir_lowering=False)

    # ---- I/O ----
    xt = nc.declare_dram_parameter("xt", [H, T], f32, isOutput=False)
    x = nc.declare_dram_parameter("x", [T, H], f32, isOutput=False)
    gw = nc.declare_dram_parameter("gw", [H, E], f32, isOutput=False)
    w1 = nc.declare_dram_parameter("w1", [H, II], f32r, isOutput=False)
    w3 = nc.declare_dram_parameter("w3", [H, II], f32r, isOutput=False)
    w2 = nc.declare_dram_parameter("w2", [II, H], f32r, isOutput=False)
    oh = nc.declare_dram_parameter("oh", [P, NT * E], f32, isOutput=False)
    tokid = nc.declare_dram_parameter("tokid", [P, NT], f32, isOutput=False)
    slotg_d = nc.declare_dram_parameter("slotg", [16, cf], f32, isOutput=False)
    ident = nc.declare_dram_parameter("ident", [P, P], f32, isOutput=False)

    o_yt = nc.declare_dram_parameter("o_yt", [H, cap], f32, isOutput=True)
    o_idx = nc.declare_dram_parameter("o_idx", [cap], i32, isOutput=True)
    o_cnt = nc.declare_dram_parameter("o_cnt", [1, 1], u32, isOutput=True)

    d_idx = nc.dram_tensor("d_idx", [cap], i32)
    d_cf = nc.dram_tensor("d_cf", [cap], f32)

    with TileContext(nc) as tc:
        with (
            tc.tile_pool(name="sb", bufs=1) as sb,
            tc.tile_pool(name="sbw", bufs=2) as sbw,
            tc.tile_pool(name="psum", bufs=2, space="PSUM") as psg,
        ):
            # ---- constants ----
            idt = sb.tile([P, P], f32, tag="idt")
            nc.sync.dma_start(out=idt[:], in_=ident[:])
            oh_sb = sb.tile([P, NT * E], f32, tag="oh")
            nc.sync.dma_start(out=oh_sb[:], in_=oh[:])
            tk = sb.tile([P, NT], f32, tag="tk")
            nc.sync.dma_start(out=tk[:], in_=tokid[:])
            slotg = sb.tile([16, cf], f32, tag="slotg")
            nc.sync.dma_start(out=slotg[:], in_=slotg_d[:])
            gw_sb = sb.tile([P, HC * E], f32, tag="gw")
            nc.sync.dma_start(
                out=gw_sb[:].rearrange("p (hc e) -> p hc e", e=E),
                in_=gw[:].rearrange("(hc p) e -> p hc e", p=P),
            )

            # ---- A. router: logitsT [8, 2048] = gw.T @ x.T, fp32 ----
            logitsT = sb.tile([E, T], f32, tag="logitsT")
            ps_l = [psg.tile([E, 512], f32, tag=f"mm{ng}", name=f"psl{ng}") for ng in range(4)]
            for hc in range(HC):
                xt_sl = sbw.tile([P, T], f32, tag="xt")
                nc.sync.dma_start(out=xt_sl[:], in_=xt[hc * P:(hc + 1) * P, :])
                for ng in range(4):
                    nc.tensor.matmul(
                        out=ps_l[ng][:],
                        lhsT=gw_sb[:, hc * E:(hc + 1) * E],
                        rhs=xt_sl[:, ng * 512:(ng + 1) * 512],
                        start=(hc == 0),
                        stop=(hc == HC - 1),
                    )
            for ng in range(4):
                nc.vector.tensor_copy(
                    out=logitsT[:, ng * 512:(ng + 1) * 512], in_=ps_l[ng][:]
                )

            # ---- B. transpose logitsT -> l_all [128, (16, 8)] ----
            l_all = sb.tile([P, NT * E], f32, tag="l_all")
            for ci in range(NT):
                tp = psg.tile([P, E], f32, tag="mm3")
                nc.tensor.transpose(
                    out=tp[:],
                    in_=logitsT[:, ci * P:(ci + 1) * P],
                    identity=idt[0:E, 0:E],
                )
                nc.vector.tensor_copy(out=l_all[:, ci * E:(ci + 1) * E], in_=tp[:])

            # ---- C. top-2 + coef ----
            l3 = l_all[:].rearrange("p (t e) -> p t e", e=E)
            m1a = sb.tile([P, NT, 4], f32, tag="m1a")
            m2a = sb.tile([P, NT, 4], f32, tag="m2a")
            nc.vector.tensor_tensor(out=m1a[:], in0=l3[:, :, 0::2], in1=l3[:, :, 1::2], op=OP.max)
            nc.vector.tensor_tensor(out=m2a[:], in0=l3[:, :, 0::2], in1=l3[:, :, 1::2], op=OP.min)
            m1b = sb.tile([P, NT, 2], f32, tag="m1b")
            m2b = sb.tile([P, NT, 2], f32, tag="m2b")
            tmin = sb.tile([P, NT, 2], f32, tag="tmin")
            nc.vector.tensor_tensor(out=m1b[:], in0=m1a[:, :, 0::2], in1=m1a[:, :, 1::2], op=OP.max)
            nc.vector.tensor_tensor(out=tmin[:], in0=m1a[:, :, 0::2], in1=m1a[:, :, 1::2], op=OP.min)
            nc.vector.tensor_tensor(out=m2b[:], in0=m2a[:, :, 0::2], in1=m2a[:, :, 1::2], op=OP.max)
            nc.vector.tensor_tensor(out=m2b[:], in0=m2b[:], in1=tmin[:], op=OP.max)
            m1 = sb.tile([P, NT, 1], f32, tag="m1")
            m2 = sb.tile([P, NT, 1], f32, tag="m2")
            tmin2 = sb.tile([P, NT, 1], f32, tag="tmin2")
            nc.vector.tensor_tensor(out=m1[:], in0=m1b[:, :, 0:1], in1=m1b[:, :, 1:2], op=OP.max)
            nc.vector.tensor_tensor(out=tmin2[:], in0=m1b[:, :, 0:1], in1=m1b[:, :, 1:2], op=OP.min)
            nc.vector.tensor_tensor(out=m2[:], in0=m2b[:, :, 0:1], in1=m2b[:, :, 1:2], op=OP.max)
            nc.vector.tensor_tensor(out=m2[:], in0=m2[:], in1=tmin2[:], op=OP.max)

            dq = sb.tile([P, NT], f32, tag="dq")
            nc.vector.tensor_sub(out=dq[:], in0=m2[:, :, 0], in1=m1[:, :, 0])
            q = sb.tile([P, NT], f32, tag="q")
            nc.scalar.activation(out=q[:], in_=dq[:], func=AF.Exp)
            s = sb.tile([P, NT], f32, tag="s")
            nc.vector.tensor_scalar_add(s[:], q[:], 1.0)
            wt1 = sb.tile([P, NT], f32, tag="wt1")
            nc.vector.reciprocal(wt1[:], s[:])
            wt2 = sb.tile([P, NT], f32, tag="wt2")
            nc.vector.tensor_mul(out=wt2[:], in0=q[:], in1=wt1[:])

            le_m = sb.tile([P, NT, E], f32, tag="lem")
            nc.vector.tensor_mul(
                out=le_m[:], in0=l3[:], in1=oh_sb[:].rearrange("p (t e) -> p t e", e=E)
            )
            le = sb.tile([P, NT], f32, tag="le")
            nc.vector.reduce_sum(
                out=le[:].rearrange("p (t o) -> p t o", o=1),
                in_=le_m[:],
                axis=mybir.AxisListType.X,
            )

            eq1 = sb.tile([P, NT], f32, tag="eq1")
            eq2 = sb.tile([P, NT], f32, tag="eq2")
            nc.vector.tensor_tensor(out=eq1[:], in0=le[:], in1=m1[:, :, 0], op=OP.is_equal)
            nc.vector.tensor_tensor(out=eq2[:], in0=le[:], in1=m2[:, :, 0], op=OP.is_equal)
            coef = sb.tile([P, NT], f32, tag="coef")
            t1 = sb.tile([P, NT], f32, tag="t1")
            nc.vector.tensor_mul(out=coef[:], in0=eq1[:], in1=wt1[:])
            nc.vector.tensor_mul(out=t1[:], in0=eq2[:], in1=wt2[:])
            nc.vector.tensor_add(out=coef[:], in0=coef[:], in1=t1[:])
            selm = sb.tile([P, NT], f32, tag="selm")
            nc.vector.tensor_add(out=selm[:], in0=eq1[:], in1=eq2[:])

            # ---- D. compaction ----
            enc = sb.tile([P, 2 * NT], f32, tag="enc")
            tkp = sb.tile([P, NT], f32, tag="tkp")
            nc.vector.tensor_scalar_add(tkp[:], tk[:], 1.0)
            nc.vector.tensor_mul(out=enc[:, 0:NT], in0=tkp[:], in1=selm[:])
            nc.vector.tensor_scalar_sub(enc[:, 0:NT], enc[:, 0:NT], 1.0)
            cfp = sb.tile([P, NT], f32, tag="cfp")
            nc.vector.tensor_scalar_add(cfp[:], coef[:], 1.0)
            nc.vector.tensor_mul(out=enc[:, NT:2 * NT], in0=cfp[:], in1=selm[:])
            nc.vector.tensor_scalar_sub(enc[:, NT:2 * NT], enc[:, NT:2 * NT], 1.0)

            enc_t = sb.tile([NT, P], f32, tag="enc_t")
            enc_t2 = sb.tile([NT, P], f32, tag="enc_t2")
            tp1 = psg.tile([NT, P], f32, tag="mm3")
            nc.tensor.transpose(out=tp1[:], in_=enc[:, 0:NT], identity=idt[:])
            nc.vector.tensor_copy(out=enc_t[:], in_=tp1[:])
            tp2 = psg.tile([NT, P], f32, tag="mm3")
            nc.tensor.transpose(out=tp2[:], in_=enc[:, NT:2 * NT], identity=idt[:])
            nc.vector.tensor_copy(out=enc_t2[:], in_=tp2[:])

            sg_idx = sb.tile([16, P], f32, tag="sgidx")
            sg_cf = sb.tile([16, P], f32, tag="sgcf")
            nf1 = sb.tile([1, 1], u32, tag="nf1")
            nf2 = sb.tile([1, 1], u32, tag="nf2")
            nc.gpsimd.sparse_gather(out=sg_idx[:], in_=enc_t[:], num_found=nf1[:])
            nc.gpsimd.sparse_gather(out=sg_cf[:], in_=enc_t2[:], num_found=nf2[:])
            nc.sync.dma_start(out=o_cnt[:], in_=nf1[:])

            # valid-slot mask (sparse_gather tail is garbage on HW)
            nf_f = sb.tile([1, 1], f32, tag="nff")
            nc.vector.tensor_copy(out=nf_f[:], in_=nf1[:])
            ones16 = sb.tile([1, 16], f32, tag="ones16")
            nc.vector.memset(ones16[:], 1.0)
            nf_b_ps = psg.tile([16, 1], f32, tag="mm3")
            nc.tensor.matmul(out=nf_b_ps[:], lhsT=ones16[:], rhs=nf_f[:], start=True, stop=True)
            nf_b = sb.tile([16, 1], f32, tag="nfbs")
            nc.vector.tensor_copy(out=nf_b[:], in_=nf_b_ps[:])
            slot_mask = sb.tile([16, cf], u8, tag="slotm")
            nc.vector.tensor_tensor(
                out=slot_mask[:], in0=slotg[:],
                in1=nf_b[:].to_broadcast([16, cf]), op=OP.is_lt,
            )
            idx_f = sb.tile([16, cf], f32, tag="idxf")
            nc.vector.memset(idx_f[:], 0.0)
            nc.vector.copy_predicated(out=idx_f[:], mask=slot_mask[:], data=sg_idx[:, 0:cf])
            idx_i = sb.tile([16, cf], i32, tag="idxi")
            nc.vector.tensor_copy(out=idx_i[:], in_=idx_f[:])
            cf_c = sb.tile([16, cf], f32, tag="cfc")
            nc.vector.memset(cf_c[:], 0.0)
            nc.vector.copy_predicated(out=cf_c[:], mask=slot_mask[:], data=sg_cf[:, 0:cf])

            # free-major relayout through DRAM
            nc.sync.dma_start(out=d_idx[:].rearrange("(f p) -> p f", p=16), in_=idx_i[:])
            nc.sync.dma_start(out=d_cf[:].rearrange("(f p) -> p f", p=16), in_=cf_c[:])
            nc.sync.dma_start(out=o_idx[:], in_=d_idx[:])
            idx_sb = sb.tile([P, ntt], i32, tag="idxsb")
            nc.sync.dma_start(out=idx_sb[:], in_=d_idx[:].rearrange("(k p) -> p k", p=P))
            cf_row = sb.tile([1, cap], f32, tag="cfrow")
            nc.sync.dma_start(out=cf_row[:], in_=d_cf[:].rearrange("(o c) -> o c", o=1))

            # ---- F. coef broadcast [128, cap] ----
            onesP = sb.tile([1, P], f32, tag="onesP")
            nc.vector.memset(onesP[:], 1.0)
            cbc = sb.tile([P, cap], f32, tag="cbc")
            for g in range(ngrp):
                cb_ps = psg.tile([P, grp], f32, tag="mm2")
                nc.tensor.matmul(
                    out=cb_ps[:], lhsT=onesP[:],
                    rhs=cf_row[:, g * grp:(g + 1) * grp], start=True, stop=True,
                )
                nc.vector.tensor_copy(out=cbc[:, g * grp:(g + 1) * grp], in_=cb_ps[:])

            # ---- E. gather selected token rows + transpose to [H, cap] ----
            xgT = [sb.tile([P, cap], f32r, tag=f"xgT{hc}", name=f"xgT{hc}") for hc in range(HC)]
            for k in range(ntt):
                xg = sbw.tile([P, H], f32, tag="xg")
                nc.gpsimd.indirect_dma_start(
                    out=xg[:], out_offset=None,
                    in_=x[:],
                    in_offset=bass.IndirectOffsetOnAxis(ap=idx_sb[:, k:k + 1], axis=0),
                )
                for hc in range(HC):
                    tpx = psg.tile([P, P], f32, tag="mm3")
                    nc.tensor.transpose(
                        out=tpx[:], in_=xg[:, hc * P:(hc + 1) * P], identity=idt[:]
                    )
                    nc.vector.tensor_copy(
                        out=xgT[hc][:, k * P:(k + 1) * P], in_=tpx[:]
                    )

            # ---- G. h1 = x@w1, h3 = x@w3 (transposed), fused silu*mul ----
            actT = [sb.tile([P, cap], f32r, tag=f"actT{ic}", name=f"actT{ic}") for ic in range(IC)]
            for ic in range(IC):
                w1_sl = sbw.tile([P, H], f32r, tag="w1sl")
                nc.sync.dma_start(
                    out=w1_sl[:].rearrange("p (hc i) -> p hc i", i=P),
                    in_=w1[:, ic * P:(ic + 1) * P].rearrange("(hc p) i -> p hc i", p=P),
                )
                w3_sl = sbw.tile([P, H], f32r, tag="w3sl")
                nc.sync.dma_start(
                    out=w3_sl[:].rearrange("p (hc i) -> p hc i", i=P),
                    in_=w3[:, ic * P:(ic + 1) * P].rearrange("(hc p) i -> p hc i", p=P),
                )
                for g in range(ngrp):
                    gs = slice(g * grp, (g + 1) * grp)
                    ps1 = psg.tile([P, grp], f32, tag="mm0")
                    ps3 = psg.tile([P, grp], f32, tag="mm1")
                    for hc in range(HC):
                        nc.tensor.matmul(
                            out=ps1[:],
                            lhsT=w1_sl[:, hc * P:(hc + 1) * P],
                            rhs=xgT[hc][:, gs],
                            start=(hc == 0), stop=(hc == HC - 1),
                        )
                    for hc in range(HC):
                        nc.tensor.matmul(
                            out=ps3[:],
                            lhsT=w3_sl[:, hc * P:(hc + 1) * P],
                            rhs=xgT[hc][:, gs],
                            start=(hc == 0), stop=(hc == HC - 1),
                        )
                    sl = sbw.tile([P, grp], f32, tag="silu")
                    nc.scalar.activation(out=sl[:], in_=ps1[:], func=AF.Silu)
                    nc.vector.tensor_mul(out=actT[ic][:, gs], in0=sl[:], in1=ps3[:])

            # ---- H. yT = (act @ w2).T * coef ----
            for hc in range(HC):
                w2_sl = sbw.tile([P, II], f32r, tag="w2sl")
                nc.sync.dma_start(
                    out=w2_sl[:].rearrange("p (ic h) -> p ic h", h=P),
                    in_=w2[:, hc * P:(hc + 1) * P].rearrange("(ic p) h -> p ic h", p=P),
                )
                for g in range(ngrp):
                    gs = slice(g * grp, (g + 1) * grp)
                    pso = psg.tile([P, grp], f32, tag="mm2")
                    for ic in range(IC):
                        nc.tensor.matmul(
                            out=pso[:],
                            lhsT=w2_sl[:, ic * P:(ic + 1) * P],
                            rhs=actT[ic][:, gs],
                            start=(ic == 0), stop=(ic == IC - 1),
                        )
                    yt_sb = sbw.tile([P, grp], f32, tag="yt")
                    nc.vector.tensor_mul(out=yt_sb[:], in0=pso[:], in1=cbc[:, gs])
                    nc.sync.dma_start(
                        out=o_yt[hc * P:(hc + 1) * P, gs], in_=yt_sb[:]
                    )

    nc.compile()
    return nc


def _get_built(cap):
    if cap not in _build_cache:
        _build_cache[cap] = _build(cap)
    return _build_cache[cap]


def _run(cap, hs, gate_w, w1s, w2s, w3s, trace=False):
    from concourse.bass_utils import run_bass_kernel_spmd

    nc = _get_built(cap)

    xt_np = np.ascontiguousarray(hs.T)
    oh_base = np.zeros((P, NT, E), np.float32)
    tokid_np = (np.arange(NT)[None, :] * P + np.arange(P)[:, None]).astype(np.float32)
    slotg_np = (np.arange(cap // 16)[None, :] * 16 + np.arange(16)[:, None]).astype(np.float32)
    ident_np = np.eye(P, dtype=np.float32)

    in_maps = []
    for c in range(NCORES):
        oh_c = oh_base.copy()
        oh_c[:, :, c] = 1.0
        in_maps.append({
            "xt": xt_np,
            "x": hs,
            "gw": gate_w,
            "w1": np.ascontiguousarray(w1s[c]),
            "w3": np.ascontiguousarray(w3s[c]),
            "w2": np.ascontiguousarray(w2s[c]),
            "oh": oh_c.reshape(P, NT * E),
            "tokid": tokid_np,
            "slotg": slotg_np,
            "ident": ident_np,
        })

    res = run_bass_kernel_spmd(nc, in_maps, list(range(NCORES)), trace=trace)
    return res


def kernel(hidden_states, gate_w, w1s, w2s, w3s, _trace=False, _cap=640):
    hs = np.ascontiguousarray(np.asarray(hidden_states, dtype=np.float32))
    gate_w = np.ascontiguousarray(np.asarray(gate_w, dtype=np.float32))
    w1s = np.asarray(w1s, dtype=np.float32)
    w2s = np.asarray(w2s, dtype=np.float32)
    w3s = np.asarray(w3s, dtype=np.float32)

    cap = _cap
    while True:
        res = _run(cap, hs, gate_w, w1s, w2s, w3s, trace=_trace)
        counts = [int(res.results[c]["o_cnt"].ravel()[0]) for c in range(NCORES)]
        if max(counts) <= cap:
            break
        # capacity overflow (won't happen for sane routing): rebuild bigger
        cap = 2048 if max(counts) > 1024 else 1024

    out = np.zeros((T, H), dtype=np.float32)
    for c in range(NCORES):
        r = res.results[c]
        cnt = counts[c]
        idx = r["o_idx"][:cnt]
        y = np.ascontiguousarray(r["o_yt"].T[:cnt])
        out[idx] += y
    kernel._last_results = res
    return out


# revision 8
# speedup vs baseline: 1.1644x; 1.1644x over previous
"""MoE (8 experts, top-2) Trainium2 kernel, expert-parallel across 8 NeuronCores.

Strategy:
  - Each core owns one expert (weights sharded along the expert axis; gate
    replicated). Everything data-dependent runs on device:
      * router logits (fp32 matmul), top-2 + renormalized gate weights (DVE)
      * per-expert token compaction (gpsimd sparse_gather)
      * token dispatch (indirect DMA gather of selected token rows)
      * expert MLP GEMMs in fp32r (silu(x@w1) * (x@w3)) @ w2, scaled by the
        gate coefficient
  - Each core returns its expert's (transposed) token outputs + the compacted
    token index list; the host scatter-adds the 8 partial outputs (the
    "combine" / unshard step).
"""
import sys

sys.path.insert(0, "/opt/trn_rl_repo")

import numpy as np

T, H, II, E = 2048, 1024, 4096, 8
P = 128
NT = T // P          # 16 token tiles
HC = H // P          # 8 hidden chunks
IC = II // P         # 32 intermediate chunks
NCORES = 8

_build_cache = {}


def _build(cap):
    """Build + schedule the per-core Tile kernel for token capacity `cap`."""
    import concourse.bass as bass
    import concourse.bacc as bacc
    import concourse.mybir as mybir
    from concourse.tile import TileContext

    f32 = mybir.dt.float32
    f32r = mybir.dt.float32r
    i32 = mybir.dt.int32
    u32 = mybir.dt.uint32
    u8 = mybir.dt.uint8
    bf16 = mybir.dt.bfloat16
    AF = mybir.ActivationFunctionType
    OP = mybir.AluOpType

    _grp_table = {512: 2, 640: 2, 768: 2, 896: 2, 1024: 2, 1536: 3, 2048: 4}
    assert cap in _grp_table, cap
    ngrp = _grp_table[cap]
    grp = cap // ngrp    # token group size per PSUM accumulation (<=512, >=256)
    cf = cap // 16       # free cols of [16, cf] compacted layout
    ntt = cap // P       # gather tiles

    nc = bacc.Bacc("TRN2", target_bir_lowering=False)

    # ---- I/O ----
    xt = nc.declare_dram_parameter("xt", [H, T], f32, isOutput=False)
    x = nc.declare_dram_parameter("x", [T, H], bf16, isOutput=False)
    gw = nc.declare_dram_parameter("gw", [H, E], f32, isOutput=False)
    w1 = nc.declare_dram_parameter("w1", [H, II], bf16, isOutput=False)
    w3 = nc.declare_dram_parameter("w3", [H, II], bf16, isOutput=False)
    w2 = nc.declare_dram_parameter("w2", [II, H], bf16, isOutput=False)
    oh = nc.declare_dram_parameter("oh", [P, NT * E], f32, isOutput=False)
    tokid = nc.declare_dram_parameter("tokid", [P, NT], f32, isOutput=False)
    slotg_d = nc.declare_dram_parameter("slotg", [16, cf], f32, isOutput=False)
    ident = nc.declare_dram_parameter("ident", [P, P], f32, isOutput=False)

    o_yt = nc.declare_dram_parameter("o_yt", [H, cap], f32, isOutput=True)
    o_idx = nc.declare_dram_parameter("o_idx", [cap], i32, isOutput=True)
    o_cnt = nc.declare_dram_parameter("o_cnt", [1, 1], u32, isOutput=True)

    d_idx = nc.dram_tensor("d_idx", [cap], i32)
    d_cf = nc.dram_tensor("d_cf", [cap], f32)

    with TileContext(nc) as tc:
        with (
            tc.tile_pool(name="sb", bufs=1) as sb,
            tc.tile_pool(name="sbw", bufs=2) as sbw,
            tc.tile_pool(name="psum", bufs=2, space="PSUM") as psg,
        ):
            # ---- constants ----
            idt = sb.tile([P, P], f32, tag="idt")
            nc.sync.dma_start(out=idt[:], in_=ident[:])
            oh_sb = sb.tile([P, NT * E], f32, tag="oh")
            nc.sync.dma_start(out=oh_sb[:], in_=oh[:])
            tk = sb.tile([P, NT], f32, tag="tk")
            nc.sync.dma_start(out=tk[:], in_=tokid[:])
            slotg = sb.tile([16, cf], f32, tag="slotg")
            nc.sync.dma_start(out=slotg[:], in_=slotg_d[:])
            gw_sb = sb.tile([P, HC * E], f32, tag="gw")
            nc.sync.dma_start(
                out=gw_sb[:].rearrange("p (hc e) -> p hc e", e=E),
                in_=gw[:].rearrange("(hc p) e -> p hc e", p=P),
            )

            # ---- A. router: logitsT [8, 2048] = gw.T @ x.T, fp32 ----
            logitsT = sb.tile([E, T], f32, tag="logitsT")
            ps_l = [psg.tile([E, 512], f32, tag=f"mm{ng}", name=f"psl{ng}") for ng in range(4)]
            for hc in range(HC):
                xt_sl = sbw.tile([P, T], f32, tag="xt")
                nc.sync.dma_start(out=xt_sl[:], in_=xt[hc * P:(hc + 1) * P, :])
                for ng in range(4):
                    nc.tensor.matmul(
                        out=ps_l[ng][:],
                        lhsT=gw_sb[:, hc * E:(hc + 1) * E],
                        rhs=xt_sl[:, ng * 512:(ng + 1) * 512],
                        start=(hc == 0),
                        stop=(hc == HC - 1),
                    )
            for ng in range(4):
                nc.vector.tensor_copy(
                    out=logitsT[:, ng * 512:(ng + 1) * 512], in_=ps_l[ng][:]
                )

            # ---- B. transpose logitsT -> l_all [128, (16, 8)] ----
            l_all = sb.tile([P, NT * E], f32, tag="l_all")
            for ci in range(NT):
                tp = psg.tile([P, E], f32, tag="mm3")
                nc.tensor.transpose(
                    out=tp[:],
                    in_=logitsT[:, ci * P:(ci + 1) * P],
                    identity=idt[0:E, 0:E],
                )
                nc.vector.tensor_copy(out=l_all[:, ci * E:(ci + 1) * E], in_=tp[:])

            # ---- C. top-2 + coef ----
            l3 = l_all[:].rearrange("p (t e) -> p t e", e=E)
            m1a = sb.tile([P, NT, 4], f32, tag="m1a")
            m2a = sb.tile([P, NT, 4], f32, tag="m2a")
            nc.vector.tensor_tensor(out=m1a[:], in0=l3[:, :, 0::2], in1=l3[:, :, 1::2], op=OP.max)
            nc.vector.tensor_tensor(out=m2a[:], in0=l3[:, :, 0::2], in1=l3[:, :, 1::2], op=OP.min)
            m1b = sb.tile([P, NT, 2], f32, tag="m1b")
            m2b = sb.tile([P, NT, 2], f32, tag="m2b")
            tmin = sb.tile([P, NT, 2], f32, tag="tmin")
            nc.vector.tensor_tensor(out=m1b[:], in0=m1a[:, :, 0::2], in1=m1a[:, :, 1::2], op=OP.max)
            nc.vector.tensor_tensor(out=tmin[:], in0=m1a[:, :, 0::2], in1=m1a[:, :, 1::2], op=OP.min)
            nc.vector.tensor_tensor(out=m2b[:], in0=m2a[:, :, 0::2], in1=m2a[:, :, 1::2], op=OP.max)
            nc.vector.tensor_tensor(out=m2b[:], in0=m2b[:], in1=tmin[:], op=OP.max)
            m1 = sb.tile([P, NT, 1], f32, tag="m1")
            m2 = sb.tile([P, NT, 1], f32, tag="m2")
            tmin2 = sb.tile([P, NT, 1], f32, tag="tmin2")
            nc.vector.tensor_tensor(out=m1[:], in0=m1b[:, :, 0:1], in1=m1b[:, :, 1:2], op=OP.max)
            nc.vector.tensor_tensor(out=tmin2[:], in0=m1b[:, :, 0:1], in1=m1b[:, :, 1:2], op=OP.min)
            nc.vector.tensor_tensor(out=m2[:], in0=m2b[:, :, 0:1], in1=m2b[:, :, 1:2], op=OP.max)
            nc.vector.tensor_tensor(out=m2[:], in0=m2[:], in1=tmin2[:], op=OP.max)

            dq = sb.tile([P, NT], f32, tag="dq")
            nc.vector.tensor_sub(out=dq[:], in0=m2[:, :, 0], in1=m1[:, :, 0])
            q = sb.tile([P, NT], f32, tag="q")
            nc.scalar.activation(out=q[:], in_=dq[:], func=AF.Exp)
            s = sb.tile([P, NT], f32, tag="s")
            nc.vector.tensor_scalar_add(s[:], q[:], 1.0)
            wt1 = sb.tile([P, NT], f32, tag="wt1")
            nc.vector.reciprocal(wt1[:], s[:])
            wt2 = sb.tile([P, NT], f32, tag="wt2")
            nc.vector.tensor_mul(out=wt2[:], in0=q[:], in1=wt1[:])

            le_m = sb.tile([P, NT, E], f32, tag="lem")
            nc.vector.tensor_mul(
                out=le_m[:], in0=l3[:], in1=oh_sb[:].rearrange("p (t e) -> p t e", e=E)
            )
            le = sb.tile([P, NT], f32, tag="le")
            nc.vector.reduce_sum(
                out=le[:].rearrange("p (t o) -> p t o", o=1),
                in_=le_m[:],
                axis=mybir.AxisListType.X,
            )

            eq1 = sb.tile([P, NT], f32, tag="eq1")
            eq2 = sb.tile([P, NT], f32, tag="eq2")
            nc.vector.tensor_tensor(out=eq1[:], in0=le[:], in1=m1[:, :, 0], op=OP.is_equal)
            nc.vector.tensor_tensor(out=eq2[:], in0=le[:], in1=m2[:, :, 0], op=OP.is_equal)
            coef = sb.tile([P, NT], f32, tag="coef")
            t1 = sb.tile([P, NT], f32, tag="t1")
            nc.vector.tensor_mul(out=coef[:], in0=eq1[:], in1=wt1[:])
            nc.vector.tensor_mul(out=t1[:], in0=eq2[:], in1=wt2[:])
            nc.vector.tensor_add(out=coef[:], in0=coef[:], in1=t1[:])
            selm = sb.tile([P, NT], f32, tag="selm")
            nc.vector.tensor_add(out=selm[:], in0=eq1[:], in1=eq2[:])

            # ---- D. compaction ----
            enc = sb.tile([P, 2 * NT], f32, tag="enc")
            tkp = sb.tile([P, NT], f32, tag="tkp")
            nc.vector.tensor_scalar_add(tkp[:], tk[:], 1.0)
            nc.vector.tensor_mul(out=enc[:, 0:NT], in0=tkp[:], in1=selm[:])
            nc.vector.tensor_scalar_sub(enc[:, 0:NT], enc[:, 0:NT], 1.0)
            cfp = sb.tile([P, NT], f32, tag="cfp")
            nc.vector.tensor_scalar_add(cfp[:], coef[:], 1.0)
            nc.vector.tensor_mul(out=enc[:, NT:2 * NT], in0=cfp[:], in1=selm[:])
            nc.vector.tensor_scalar_sub(enc[:, NT:2 * NT], enc[:, NT:2 * NT], 1.0)

            enc_t = sb.tile([NT, P], f32, tag="enc_t")
            enc_t2 = sb.tile([NT, P], f32, tag="enc_t2")
            tp1 = psg.tile([NT, P], f32, tag="mm3")
            nc.tensor.transpose(out=tp1[:], in_=enc[:, 0:NT], identity=idt[:])
            nc.vector.tensor_copy(out=enc_t[:], in_=tp1[:])
            tp2 = psg.tile([NT, P], f32, tag="mm3")
            nc.tensor.transpose(out=tp2[:], in_=enc[:, NT:2 * NT], identity=idt[:])
            nc.vector.tensor_copy(out=enc_t2[:], in_=tp2[:])

            sg_idx = sb.tile([16, P], f32, tag="sgidx")
            sg_cf = sb.tile([16, P], f32, tag="sgcf")
            nf1 = sb.tile([1, 1], u32, tag="nf1")
            nf2 = sb.tile([1, 1], u32, tag="nf2")
            nc.gpsimd.sparse_gather(out=sg_idx[:], in_=enc_t[:], num_found=nf1[:])
            nc.gpsimd.sparse_gather(out=sg_cf[:], in_=enc_t2[:], num_found=nf2[:])
            nc.sync.dma_start(out=o_cnt[:], in_=nf1[:])

            # valid-slot mask (sparse_gather tail is garbage on HW)
            nf_f = sb.tile([1, 1], f32, tag="nff")
            nc.vector.tensor_copy(out=nf_f[:], in_=nf1[:])
            ones16 = sb.tile([1, 16], f32, tag="ones16")
            nc.vector.memset(ones16[:], 1.0)
            nf_b_ps = psg.tile([16, 1], f32, tag="mm3")
            nc.tensor.matmul(out=nf_b_ps[:], lhsT=ones16[:], rhs=nf_f[:], start=True, stop=True)
            nf_b = sb.tile([16, 1], f32, tag="nfbs")
            nc.vector.tensor_copy(out=nf_b[:], in_=nf_b_ps[:])
            slot_mask = sb.tile([16, cf], u8, tag="slotm")
            nc.vector.tensor_tensor(
                out=slot_mask[:], in0=slotg[:],
                in1=nf_b[:].to_broadcast([16, cf]), op=OP.is_lt,
            )
            idx_f = sb.tile([16, cf], f32, tag="idxf")
            nc.vector.memset(idx_f[:], 0.0)
            nc.vector.copy_predicated(out=idx_f[:], mask=slot_mask[:], data=sg_idx[:, 0:cf])
            idx_i = sb.tile([16, cf], i32, tag="idxi")
            nc.vector.tensor_copy(out=idx_i[:], in_=idx_f[:])
            cf_c = sb.tile([16, cf], f32, tag="cfc")
            nc.vector.memset(cf_c[:], 0.0)
            nc.vector.copy_predicated(out=cf_c[:], mask=slot_mask[:], data=sg_cf[:, 0:cf])

            # free-major relayout through DRAM
            nc.sync.dma_start(out=d_idx[:].rearrange("(f p) -> p f", p=16), in_=idx_i[:])
            nc.sync.dma_start(out=d_cf[:].rearrange("(f p) -> p f", p=16), in_=cf_c[:])
            nc.sync.dma_start(out=o_idx[:], in_=d_idx[:])
            idx_sb = sb.tile([P, ntt], i32, tag="idxsb")
            nc.sync.dma_start(out=idx_sb[:], in_=d_idx[:].rearrange("(k p) -> p k", p=P))
            cf_row = sb.tile([1, cap], f32, tag="cfrow")
            nc.sync.dma_start(out=cf_row[:], in_=d_cf[:].rearrange("(o c) -> o c", o=1))

            idtb = sb.tile([P, P], bf16, tag="idtb")
            nc.vector.tensor_copy(out=idtb[:], in_=idt[:])

            # ---- F. coef broadcast [128, cap] ----
            onesP = sb.tile([1, P], f32, tag="onesP")
            nc.vector.memset(onesP[:], 1.0)
            cbc = sb.tile([P, cap], f32, tag="cbc")
            for g in range(ngrp):
                cb_ps = psg.tile([P, grp], f32, tag="mm2")
                nc.tensor.matmul(
                    out=cb_ps[:], lhsT=onesP[:],
                    rhs=cf_row[:, g * grp:(g + 1) * grp], start=True, stop=True,
                )
                nc.vector.tensor_copy(out=cbc[:, g * grp:(g + 1) * grp], in_=cb_ps[:])

            # ---- E. gather selected token rows + transpose to [H, cap] ----
            xgT = [sb.tile([P, cap], bf16, tag=f"xgT{hc}", name=f"xgT{hc}") for hc in range(HC)]
            for k in range(ntt):
                xg = sbw.tile([P, H], bf16, tag="xg")
                nc.gpsimd.indirect_dma_start(
                    out=xg[:], out_offset=None,
                    in_=x[:],
                    in_offset=bass.IndirectOffsetOnAxis(ap=idx_sb[:, k:k + 1], axis=0),
                )
                for hc in range(HC):
                    tpx = psg.tile([P, P], bf16, tag="mm3")
                    nc.tensor.transpose(
                        out=tpx[:], in_=xg[:, hc * P:(hc + 1) * P], identity=idtb[:]
                    )
                    nc.vector.tensor_copy(
                        out=xgT[hc][:, k * P:(k + 1) * P], in_=tpx[:]
                    )

            # ---- G. h1 = x@w1, h3 = x@w3 (transposed), fused silu*mul ----
            actT = [sb.tile([P, cap], bf16, tag=f"actT{ic}", name=f"actT{ic}") for ic in range(IC)]
            for ic in range(IC):
                w1_sl = sbw.tile([P, H], bf16, tag="w1sl")
                nc.scalar.dma_start(
                    out=w1_sl[:].rearrange("p (hc i) -> p hc i", i=P),
                    in_=w1[:, ic * P:(ic + 1) * P].rearrange("(hc p) i -> p hc i", p=P),
                )
                w3_sl = sbw.tile([P, H], bf16, tag="w3sl")
                nc.scalar.dma_start(
                    out=w3_sl[:].rearrange("p (hc i) -> p hc i", i=P),
                    in_=w3[:, ic * P:(ic + 1) * P].rearrange("(hc p) i -> p hc i", p=P),
                )
                for g in range(ngrp):
                    gs = slice(g * grp, (g + 1) * grp)
                    ps1 = psg.tile([P, grp], f32, tag="mm0")
                    ps3 = psg.tile([P, grp], f32, tag="mm1")
                    for hc in range(HC):
                        nc.tensor.matmul(
                            out=ps1[:],
                            lhsT=w1_sl[:, hc * P:(hc + 1) * P],
                            rhs=xgT[hc][:, gs],
                            start=(hc == 0), stop=(hc == HC - 1),
                        )
                    for hc in range(HC):
                        nc.tensor.matmul(
                            out=ps3[:],
                            lhsT=w3_sl[:, hc * P:(hc + 1) * P],
                            rhs=xgT[hc][:, gs],
                            start=(hc == 0), stop=(hc == HC - 1),
                        )
                    sl = sbw.tile([P, grp], f32, tag="silu")
                    nc.scalar.activation(out=sl[:], in_=ps1[:], func=AF.Silu)
                    nc.vector.tensor_mul(out=actT[ic][:, gs], in0=sl[:], in1=ps3[:])

            # ---- H. yT = (act @ w2).T * coef ----
            for hc in range(HC):
                w2_sl = sbw.tile([P, II], bf16, tag="w2sl")
                nc.scalar.dma_start(
                    out=w2_sl[:].rearrange("p (ic h) -> p ic h", h=P),
                    in_=w2[:, hc * P:(hc + 1) * P].rearrange("(ic p) h -> p ic h", p=P),
                )
                for g in range(ngrp):
                    gs = slice(g * grp, (g + 1) * grp)
                    pso = psg.tile([P, grp], f32, tag="mm2")
                    for ic in range(IC):
                        nc.tensor.matmul(
                            out=pso[:],
                            lhsT=w2_sl[:, ic * P:(ic + 1) * P],
                            rhs=actT[ic][:, gs],
                            start=(ic == 0), stop=(ic == IC - 1),
                        )
                    yt_sb = sbw.tile([P, grp], f32, tag="yt")
                    nc.vector.tensor_mul(out=yt_sb[:], in0=pso[:], in1=cbc[:, gs])
                    nc.sync.dma_start(
                        out=o_yt[hc * P:(hc + 1) * P, gs], in_=yt_sb[:]
                    )

    nc.compile()
    return nc


def _get_built(cap):
    if cap not in _build_cache:
        _build_cache[cap] = _build(cap)
    return _build_cache[cap]


def _run(cap, hs, gate_w, w1s, w2s, w3s, trace=False):
    import ml_dtypes
    from concourse.bass_utils import run_bass_kernel_spmd

    nc = _get_built(cap)

    bf = ml_dtypes.bfloat16
    xt_np = np.ascontiguousarray(hs.T)
    x_bf = np.ascontiguousarray(hs.astype(bf))
    oh_base = np.zeros((P, NT, E), np.float32)
    tokid_np = (np.arange(NT)[None, :] * P + np.arange(P)[:, None]).astype(np.float32)
    slotg_np = (np.arange(cap // 16)[None, :] * 16 + np.arange(16)[:, None]).astype(np.float32)
    ident_np = np.eye(P, dtype=np.float32)

    in_maps = []
    for c in range(NCORES):
        oh_c = oh_base.copy()
        oh_c[:, :, c] = 1.0
        in_maps.append({
            "xt": xt_np,
            "x": x_bf,
            "gw": gate_w,
            "w1": np.ascontiguousarray(w1s[c].astype(bf)),
            "w3": np.ascontiguousarray(w3s[c].astype(bf)),
            "w2": np.ascontiguousarray(w2s[c].astype(bf)),
            "oh": oh_c.reshape(P, NT * E),
            "tokid": tokid_np,
            "slotg": slotg_np,
            "ident": ident_np,
        })

    res = run_bass_kernel_spmd(nc, in_maps, list(range(NCORES)), trace=trace)
    return res


def kernel(hidden_states, gate_w, w1s, w2s, w3s, _trace=False, _cap=640):
    hs = np.ascontiguousarray(np.asarray(hidden_states, dtype=np.float32))
    gate_w = np.ascontiguousarray(np.asarray(gate_w, dtype=np.float32))
    w1s = np.asarray(w1s, dtype=np.float32)
    w2s = np.asarray(w2s, dtype=np.float32)
    w3s = np.asarray(w3s, dtype=np.float32)

    cap = _cap
    while True:
        res = _run(cap, hs, gate_w, w1s, w2s, w3s, trace=_trace)
        counts = [int(res.results[c]["o_cnt"].ravel()[0]) for c in range(NCORES)]
        if max(counts) <= cap:
            break
        # capacity overflow (won't happen for sane routing): rebuild bigger
        cap = 2048 if max(counts) > 1024 else 1024

    out = np.zeros((T, H), dtype=np.float32)
    for c in range(NCORES):
        r = res.results[c]
        cnt = counts[c]
        idx = r["o_idx"][:cnt]
        y = np.ascontiguousarray(r["o_yt"].T[:cnt])
        out[idx] += y
    kernel._last_results = res
    return out
